# revision 13
# baseline (speedup 1.0000x reference)
"""GQA attention (16 Q heads / 4 KV heads, RoPE, n=2048, d=64) on 8 trn2 cores.

Sharding: core c = (batch b=c//4, kv-group j=c%4). Each core owns 4 query
heads sharing one KV head, computes its partial output projection
(O_heads @ Wo_rows), and the host sums the 4 partials per batch.

v2 design (exp-bound pipeline):
  - fp16 activations/weights everywhere (PE 1 cycle/row, better precision
    than bf16, enables DVE 2x/4x modes on all-SBUF ops).
  - Phase A: KV+Q projections -> stage RAW q/k in SBUF; RoPE applied
    just-in-time per attention chunk from SBUF (DVE), so projections and
    ropes pipeline and attention starts early.
  - Phase B: 8 chunks (head, 1024-query half). Per key block: S matmul
    (2x 512-col) -> exp on ACT engine [128,1024] -> PV accumulate.
    Scalar/ACT exp is the roofline (~131k cols * 0.83ns). PSUM: S pool
    2x2 banks + PO pool 2x2 banks = 8.
  - Softmax denominators via ones-column of V_aug land on PSUM row 64;
    normalized via DMA-transpose -> batched reciprocal [128, 8] ->
    DMA-back -> broadcast multiply (no single-partition reciprocals).
  - Phase C: output projection, fp16 output DMA streamed per tile.
"""

import os
import sys
import functools

import numpy as np

sys.path.insert(0, "/opt/trn_rl_repo")

import concourse.bass as bass  # noqa: E402
import concourse.bacc as bacc  # noqa: E402
import concourse.tile as tile  # noqa: E402
import concourse.mybir as mybir  # noqa: E402
from concourse.masks import make_identity  # noqa: E402

F32 = mybir.dt.float32
FP16 = mybir.dt.float16
EXP = mybir.ActivationFunctionType.Exp

B, N, DIM = 2, 2048, 1024
HEADS, KVH, D = 16, 4, 64
HPC = HEADS // KVH          # q heads per core = 4
SCALE = D ** -0.5           # 1/8
NKB = N // 128              # 16 key blocks
NDB = DIM // 128            # 8 contraction blocks for projections
NCH = 8                     # attention chunks: (head, half) -> 1024 queries

LAST_RESULTS = {}           # test.py introspection


def build_kernel(nc, tc, io):
    from contextlib import ExitStack

    xt, wq, wkv, wo = io["xt"], io["wq"], io["wkv"], io["wo"]
    cost, sincat, out = io["cost"], io["sincat"], io["out"]

    es = ExitStack()
    consts = es.enter_context(tc.tile_pool(name="consts", bufs=1))
    act = es.enter_context(tc.tile_pool(name="act", bufs=1))
    ropetmp = es.enter_context(tc.tile_pool(name="ropetmp", bufs=2))
    ppool = es.enter_context(tc.tile_pool(name="ppool", bufs=3))
    stpool = es.enter_context(tc.tile_pool(name="stpool", bufs=4))

    # --- constants / weights in SBUF ---
    wq_sb = consts.tile([128, NDB, 256], FP16, tag="wq")
    wkv_sb = consts.tile([128, NDB, 128], FP16, tag="wkv")
    wo_sb = consts.tile([128, 2, DIM], FP16, tag="wo")
    # cos/sin replicated over all 128 partitions; sin table stores
    # [+sin; -sin] per 64-row block so every rope mul reads SBUF inputs at
    # matching base partitions (BIR constraint for 2-SBUF-input TensorTensor).
    cos_sb = consts.tile([128, N], FP16, tag="cos")
    sin_sb = consts.tile([128, N], FP16, tag="sin")
    id64 = consts.tile([64, 64], FP16, tag="id")
    nc.sync.dma_start(wq_sb, wq.transpose([1, 0, 2]))
    nc.sync.dma_start(wkv_sb, wkv.transpose([1, 0, 2]))
    nc.sync.dma_start(wo_sb, wo.transpose([1, 0, 2]))
    nc.sync.dma_start(cos_sb, cost)
    nc.sync.dma_start(sin_sb, sincat)
    make_identity(nc, id64)

    # --- persistent activations ---
    xt_sb = act.tile([128, NDB, N], FP16, tag="xt")
    k_raw = act.tile([64, N], FP16, tag="kraw")
    vt_sb = act.tile([64, N], FP16, tag="vt")
    q_raw = act.tile([128, 2, N], FP16, tag="qraw")       # [2 heads x 64, pack, n]
    kt_sb = act.tile([64, N], FP16, tag="kt")
    qt_sb = act.tile([64, HPC, N], FP16, tag="qt")
    vaug_sb = act.tile([128, NKB, 66], FP16, tag="vaug")  # cols 0:64 v, 64 ones
    ot_sb = act.tile([128, 2, N], FP16, tag="ot")         # normalized O^T, 2 packs
    dnrow = act.tile([1, HPC * N], F32, tag="dnrow")      # denom rows staging
    dn_sb = act.tile([128, 64], F32, tag="dn")            # denoms, transposed
    rc_sb = act.tile([128, 64], F32, tag="rc")            # 1/denoms
    rcrow = act.tile([1, HPC * N], F32, tag="rcrow")      # 1/denom back in row form
    bc_sb = act.tile([64, 2, 1024], F32, tag="bc")        # broadcast 1/denom (2 bufs)

    nc.gpsimd.memset(vaug_sb[:, :, 64:65], 1.0)

    # xt DMA ch-major so first KV projection chunk starts early.
    for ch in range(4):
        for kb in range(NDB):
            nc.sync.dma_start(
                xt_sb[:, kb, ch * 512:(ch + 1) * 512],
                xt[kb, :, ch * 512:(ch + 1) * 512],
            )

    def rope(dst, src, cols, row0):
        """dst[64, w] <- RoPE(src fp16 SBUF at base partition row0)."""
        w = cols.stop - cols.start
        t1f = ropetmp.tile([64, 1024], FP16, tag="t1", name="t1f")
        t2f = ropetmp.tile([64, 1024], FP16, tag="t2", name="t2f")
        t1, t2 = t1f[:, 0:w], t2f[:, 0:w]
        nc.vector.tensor_mul(t1, src, cos_sb[row0:row0 + 64, cols])
        nc.vector.tensor_mul(
            t2[0:32, :], src[32:64, :], sin_sb[row0 + 32:row0 + 64, cols]
        )
        nc.vector.tensor_mul(
            t2[32:64, :], src[0:32, :], sin_sb[row0:row0 + 32, cols]
        )
        nc.vector.tensor_add(dst, t1, t2)

    # ---- Phase A: projections, V transpose, k rope, raw q staging ----
    with (
        tc.tile_pool(name="psA", bufs=3, space="PSUM") as psA,
        tc.tile_pool(name="ptr", bufs=2, space="PSUM") as ptr,
    ):
        for ch in range(4):
            cols = slice(ch * 512, (ch + 1) * 512)
            pkv = psA.tile([128, 512], F32, tag="pj")
            for kb in range(NDB):
                nc.tensor.matmul(
                    pkv, wkv_sb[:, kb, :], xt_sb[:, kb, cols],
                    start=(kb == 0), stop=(kb == NDB - 1),
                )
            nc.vector.tensor_copy(k_raw[:, cols], pkv[0:64, :])
            nc.vector.tensor_copy(vt_sb[:, cols], pkv[64:128, :])
            for t in range(ch * 4, ch * 4 + 4):
                pt = ptr.tile([128, 64], FP16, tag="pjt")
                nc.tensor.transpose(
                    pt[:, 0:64], vt_sb[:, t * 128:(t + 1) * 128], id64
                )
                nc.vector.tensor_copy(vaug_sb[:, t, 0:64], pt[:, 0:64])
        # k rope (all-SBUF fp16)
        for half in range(2):
            cols = slice(half * 1024, (half + 1) * 1024)
            rope(kt_sb[:, cols], k_raw[:, cols], cols, 0)

        for pack in range(2):
            for ch in range(4):
                cols = slice(ch * 512, (ch + 1) * 512)
                pq = psA.tile([128, 512], F32, tag="pj")
                for kb in range(NDB):
                    nc.tensor.matmul(
                        pq, wq_sb[:, kb, pack * 128:(pack + 1) * 128],
                        xt_sb[:, kb, cols],
                        start=(kb == 0), stop=(kb == NDB - 1),
                    )
                nc.vector.tensor_copy(q_raw[:, pack, cols], pq)

    # ---- Phase B: attention ----
    chunks = [(h, half) for h in range(HPC) for half in range(2)]

    def jit_rope(qc):
        h, half = chunks[qc]
        pack, row0 = h // 2, (h % 2) * 64
        cols = slice(half * 1024, (half + 1) * 1024)
        rope(qt_sb[0:64, h, cols], q_raw[row0:row0 + 64, pack, cols], cols, row0)

    def drain(qc, po):
        """Normalize chunk qc's PV psum -> ot_sb (and free po)."""
        h, half = chunks[qc]
        pack, row0 = h // 2, (h % 2) * 64
        cols = slice(half * 1024, (half + 1) * 1024)
        g0 = h * 16 + half * 8
        # denom row -> SBUF staging -> refold onto 128 partitions (natural
        # order: dn[p, b] = row[p*8+b]; any bijection works since recip is
        # elementwise and the DMA back inverts it).
        row = dnrow[0:1, qc * 1024:(qc + 1) * 1024]
        nc.vector.tensor_copy(row, po[64:65, :, :].rearrange("p a b -> p (a b)"))
        nc.sync.dma_start(dn_sb[:, g0:g0 + 8], row)
        nc.vector.reciprocal(rc_sb[:, g0:g0 + 8], dn_sb[:, g0:g0 + 8])
        nc.sync.dma_start(
            rcrow[0:1, qc * 1024:(qc + 1) * 1024], rc_sb[:, g0:g0 + 8]
        )
        bc = bc_sb[:, qc % 2, :]
        nc.gpsimd.partition_broadcast(bc, rcrow[0:1, qc * 1024:(qc + 1) * 1024])
        nc.vector.tensor_mul(
            ot_sb[row0:row0 + 64, pack, cols],
            po[0:64, :, :].rearrange("p a b -> p (a b)"), bc
        )

    jit_rope(0)
    jit_rope(1)
    po_tiles = {}
    with (
        tc.tile_pool(name="psS", bufs=2, space="PSUM") as psS,
        tc.tile_pool(name="psPO", bufs=2, space="PSUM") as psPO,
    ):
        for qc, (h, half) in enumerate(chunks):
            po = psPO.tile([128, 8, 128], F32, tag="po")
            po_tiles[qc] = po
            first = True
            for kb in range(NKB):
                ps = psS.tile([128, 1024], F32, tag="s")
                for sh in range(2):
                    nc.tensor.matmul(
                        ps[:, sh * 512:(sh + 1) * 512],
                        kt_sb[:, kb * 128:(kb + 1) * 128],
                        qt_sb[0:64, h,
                              half * 1024 + sh * 512: half * 1024 + (sh + 1) * 512],
                        start=True, stop=True,
                    )
                p_t = ppool.tile([128, 1024], FP16, tag="p")
                nc.scalar.activation(p_t, ps, EXP, bias=0.0, scale=SCALE)
                for sh in range(2):
                    nc.tensor.matmul(
                        po[0:65, sh * 4:(sh + 1) * 4, :],
                        vaug_sb[:, kb, 0:65],
                        p_t[:, sh * 512:(sh + 1) * 512],
                        start=(kb == 0), stop=(kb == NKB - 1),
                        skip_group_check=True,
                    )
                if first:
                    # issue next-next chunk's rope early on the DVE queue,
                    # then the previous chunk's drain ops.
                    first = False
                    if qc + 2 < NCH:
                        jit_rope(qc + 2)
                    if qc >= 1:
                        drain(qc - 1, po_tiles.pop(qc - 1))
        drain(NCH - 1, po_tiles.pop(NCH - 1))

    # ---- Phase C: output projection out[q, :] = sum_pair O^T_pair.T @ Wo ----
    with tc.tile_pool(name="psOP", bufs=4, space="PSUM") as psOP:
        for qb in range(N // 128):
            for nchk in range(2):
                pt = psOP.tile([128, 512], F32, tag="po")
                for pair in range(2):
                    nc.tensor.matmul(
                        pt,
                        ot_sb[:, pair, qb * 128:(qb + 1) * 128],
                        wo_sb[:, pair, nchk * 512:(nchk + 1) * 512],
                        start=(pair == 0), stop=(pair == 1),
                    )
                st = stpool.tile([128, 512], FP16, tag="st")
                if (qb * 2 + nchk) % 2 == 0:
                    nc.vector.tensor_copy(st, pt)
                else:
                    nc.scalar.copy(st, pt)
                nc.sync.dma_start(
                    out[qb * 128:(qb + 1) * 128, nchk * 512:(nchk + 1) * 512], st
                )

    es.close()


def _rope_tables():
    inv_freq = 1.0 / (10000.0 ** (np.arange(0, D, 2, dtype=np.float64) / D))
    freqs = np.outer(np.arange(N, dtype=np.float64), inv_freq)  # [N, 32]
    cos_h = np.cos(freqs).T                                      # [32, N]
    sin_h = np.sin(freqs).T                                      # [32, N]
    # replicated over 128 partitions; sin blocks alternate [+sin; -sin] so
    # rope's swapped-row muls read matching base partitions (see rope()).
    cost = np.tile(cos_h, (4, 1)).astype(np.float16)
    sincat = np.concatenate(
        [sin_h, -sin_h, sin_h, -sin_h], 0
    ).astype(np.float16)
    return np.ascontiguousarray(cost), np.ascontiguousarray(sincat)


@functools.lru_cache(maxsize=1)
def _program():
    nc = bacc.Bacc(
        "TRN2", target_bir_lowering=False, debug=False, enable_asserts=False
    )
    io = {
        "xt": nc.dram_tensor("xt", [NDB, 128, N], FP16, kind="ExternalInput").ap(),
        "wq": nc.dram_tensor("wq", [NDB, 128, 256], FP16, kind="ExternalInput").ap(),
        "wkv": nc.dram_tensor("wkv", [NDB, 128, 128], FP16, kind="ExternalInput").ap(),
        "wo": nc.dram_tensor("wo", [2, 128, DIM], FP16, kind="ExternalInput").ap(),
        "cost": nc.dram_tensor("cost", [128, N], FP16, kind="ExternalInput").ap(),
        "sincat": nc.dram_tensor("sincat", [128, N], FP16, kind="ExternalInput").ap(),
        "out": nc.dram_tensor("out", [N, DIM], FP16, kind="ExternalOutput").ap(),
    }
    with tile.TileContext(nc) as tc:
        build_kernel(nc, tc, io)
    nc.compile()
    return nc


def make_in_maps(x, Wq, Wkv, Wo):
    cost, sincat = _rope_tables()
    in_maps = []
    for c in range(8):
        b, j = c // 4, c % 4
        xt = np.ascontiguousarray(x[b].T).reshape(NDB, 128, N)
        wq_c = np.ascontiguousarray(Wq[:, 256 * j:256 * (j + 1)]).reshape(
            NDB, 128, 256
        )
        wkv_c = np.ascontiguousarray(
            np.concatenate(
                [Wkv[:, 64 * j:64 * (j + 1)],
                 Wkv[:, 256 + 64 * j:256 + 64 * (j + 1)]],
                axis=1,
            )
        ).reshape(NDB, 128, 128)
        wo_c = np.ascontiguousarray(Wo[256 * j:256 * (j + 1), :]).reshape(
            2, 128, DIM
        )
        in_maps.append(
            {
                "xt": xt.astype(np.float16),
                "wq": wq_c.astype(np.float16),
                "wkv": wkv_c.astype(np.float16),
                "wo": wo_c.astype(np.float16),
                "cost": cost,
                "sincat": sincat,
            }
        )
    return in_maps


def _install_ntff_hook():
    """Register the axon NTFF profiling hook that this image's antenv lacks."""
    import types

    if "antenv.axon_hooks" in sys.modules:
        return
    try:
        sys.path.append("/root/.axon_site")
        from trn_agent_boot.trn_boot import _ntff_profile_via_ctypes

        hook = _ntff_profile_via_ctypes("/opt/axon/libaxon_pjrt.so")
    except Exception:
        hook = None
    finally:
        try:
            sys.path.remove("/root/.axon_site")
        except ValueError:
            pass
    mod = types.ModuleType("antenv.axon_hooks")
    mod.get_axon_ntff_profile_hook = lambda: hook
    mod.set_axon_ntff_profile_hook = lambda h: None
    sys.modules["antenv.axon_hooks"] = mod
    # artifact upload needs bucket credentials this container lacks
    import concourse.bass_utils as bu

    bu.upload_artifacts = lambda tmpdir: "local://" + str(tmpdir)


def kernel(x, Wq, Wkv, Wo, bo):
    from concourse.bass_utils import run_bass_kernel_spmd

    _install_ntff_hook()
    nc = _program()
    in_maps = make_in_maps(x, Wq, Wkv, Wo)
    trace = bool(os.environ.get("KERNEL_TRACE"))
    res = run_bass_kernel_spmd(
        nc, in_maps, list(range(8)), trace=trace
    )
    LAST_RESULTS["res"] = res
    full = np.zeros((B, N, DIM), np.float32)
    for c in range(8):
        full[c // 4] += res.results[c]["out"].astype(np.float32)
    full += bo.astype(np.float32)
    return full


# revision 15
# speedup vs baseline: 1.5938x; 1.5938x over previous
"""GQA attention (16 Q heads / 4 KV heads, RoPE, n=2048, d=64) on 8 trn2 cores.

Sharding: core c = (batch b=c//4, kv-group j=c%4). Each core owns 4 query
heads sharing one KV head, computes its partial output projection
(O_heads @ Wo_rows), and the host sums the 4 partials per batch.

v3 design (exp-bound pipeline, bf16):
  - bf16 matmul operands everywhere (fp16 measured ~2.7x slower on the real
    PE despite the cost model); S/PV matmuls zero-padded to 128-partition /
    128-col shapes (odd shapes also measured slow).
  - Phase A: KV + pack-0 Q projections -> stage RAW q/k in SBUF; RoPE
    applied just-in-time per attention chunk from SBUF on the DVE.
    Pack-1 Q projections are injected into attention chunks 0-3 (psS slot
    steal) to shorten the startup ramp.
  - Phase B: 8 chunks (head, 1024-query half). Per key block: S matmul
    (2x 512-col) -> exp on ACT engine [128,1024] -> PV accumulate.
    ACT exp is the roofline (~131k cols * 0.83ns = 109us + op overheads).
    PSUM: S pool 2x2 banks + PO pool 2x2 banks = 8.
  - Softmax denominators via ones-column of V_aug land on PSUM row 64;
    reciprocal'd in a [128, 8] transposed layout (DMA round trip) instead
    of single-partition reciprocals, broadcast multiply on drain.
  - Phase C: output projection, bf16 output DMA streamed per tile; host
    sums the per-core partials in fp32.
"""

import os
import sys
import functools

import numpy as np

sys.path.insert(0, "/opt/trn_rl_repo")

import concourse.bass as bass  # noqa: E402
import concourse.bacc as bacc  # noqa: E402
import concourse.tile as tile  # noqa: E402
import concourse.mybir as mybir  # noqa: E402
from concourse.masks import make_identity  # noqa: E402

F32 = mybir.dt.float32
BF16 = mybir.dt.bfloat16
EXP = mybir.ActivationFunctionType.Exp

B, N, DIM = 2, 2048, 1024
HEADS, KVH, D = 16, 4, 64
HPC = HEADS // KVH          # q heads per core = 4
SCALE = D ** -0.5           # 1/8
NKB = N // 128              # 16 key blocks
NDB = DIM // 128            # 8 contraction blocks for projections
NCH = 8                     # attention chunks: (head, half) -> 1024 queries

LAST_RESULTS = {}           # test.py introspection


def build_kernel(nc, tc, io):
    from contextlib import ExitStack

    xt, wq, wkv, wo = io["xt"], io["wq"], io["wkv"], io["wo"]
    cost, sincat, out = io["cost"], io["sincat"], io["out"]

    es = ExitStack()
    consts = es.enter_context(tc.tile_pool(name="consts", bufs=1))
    act = es.enter_context(tc.tile_pool(name="act", bufs=1))
    ropetmp = es.enter_context(tc.tile_pool(name="ropetmp", bufs=2))
    ppool = es.enter_context(tc.tile_pool(name="ppool", bufs=3))
    stpool = es.enter_context(tc.tile_pool(name="stpool", bufs=4))

    # --- constants / weights in SBUF ---
    wq_sb = consts.tile([128, NDB, 256], BF16, tag="wq")
    wkv_sb = consts.tile([128, NDB, 128], BF16, tag="wkv")
    wo_sb = consts.tile([128, 2, DIM], BF16, tag="wo")
    # cos/sin replicated over all 128 partitions; sin table stores
    # [+sin; -sin] per 64-row block so every rope mul reads its two SBUF
    # inputs at matching base partitions (BIR constraint).
    cos_sb = consts.tile([128, N], F32, tag="cos")
    sin_sb = consts.tile([128, N], F32, tag="sin")
    id64 = consts.tile([64, 64], BF16, tag="id")
    nc.sync.dma_start(wq_sb, wq.transpose([1, 0, 2]))
    nc.sync.dma_start(wkv_sb, wkv.transpose([1, 0, 2]))
    nc.sync.dma_start(wo_sb, wo.transpose([1, 0, 2]))
    nc.sync.dma_start(cos_sb, cost)
    nc.sync.dma_start(sin_sb, sincat)
    make_identity(nc, id64)

    # --- persistent activations ---
    xt_sb = act.tile([128, NDB, N], BF16, tag="xt")
    k_raw = act.tile([64, N], BF16, tag="kraw")
    vt_sb = act.tile([64, N], BF16, tag="vt")
    q_raw = act.tile([128, 2, N], BF16, tag="qraw")       # [2 heads x 64, pack, n]
    kt_sb = act.tile([128, N], BF16, tag="kt")            # rows 64:128 zero
    qt_sb = act.tile([128, HPC, N], BF16, tag="qt")       # rows 64:128 zero
    vaug_sb = act.tile([128, NKB, 128], BF16, tag="vaug")  # v | ones | zeros
    ot_sb = act.tile([128, 2, N], BF16, tag="ot")         # normalized O^T, 2 packs
    dnrow = act.tile([1, HPC * N], F32, tag="dnrow")      # denom rows staging
    dn_sb = act.tile([128, 64], F32, tag="dn")            # denoms, refolded
    rc_sb = act.tile([128, 64], F32, tag="rc")            # 1/denoms
    rcrow = act.tile([1, HPC * N], F32, tag="rcrow")      # 1/denom row form
    bc_sb = act.tile([64, 2, 1024], F32, tag="bc")        # broadcast 1/denom

    nc.gpsimd.memset(kt_sb[64:128, :], 0.0)
    nc.gpsimd.memset(qt_sb[64:128, :, :], 0.0)
    nc.gpsimd.memset(vaug_sb[:, :, 64:65], 1.0)
    nc.gpsimd.memset(vaug_sb[:, :, 65:128], 0.0)

    # xt DMA ch-major so the first KV projection chunk starts early.
    for ch in range(4):
        for kb in range(NDB):
            nc.sync.dma_start(
                xt_sb[:, kb, ch * 512:(ch + 1) * 512],
                xt[kb, :, ch * 512:(ch + 1) * 512],
            )

    def rope(dst, src, cols, row0):
        """dst[64, w] <- RoPE(src bf16 SBUF at base partition row0)."""
        w = cols.stop - cols.start
        t1f = ropetmp.tile([64, 1024], F32, tag="t1", name="t1f")
        t2f = ropetmp.tile([64, 1024], F32, tag="t2", name="t2f")
        t1, t2 = t1f[:, 0:w], t2f[:, 0:w]
        nc.vector.tensor_mul(t1, src, cos_sb[row0:row0 + 64, cols])
        nc.vector.tensor_mul(
            t2[0:32, :], src[32:64, :], sin_sb[row0 + 32:row0 + 64, cols]
        )
        nc.vector.tensor_mul(
            t2[32:64, :], src[0:32, :], sin_sb[row0:row0 + 32, cols]
        )
        nc.vector.tensor_add(dst, t1, t2)

    def qproj(psum_tile, pack, ch):
        cols = slice(ch * 512, (ch + 1) * 512)
        for kb in range(NDB):
            nc.tensor.matmul(
                psum_tile, wq_sb[:, kb, pack * 128:(pack + 1) * 128],
                xt_sb[:, kb, cols],
                start=(kb == 0), stop=(kb == NDB - 1),
            )
        nc.vector.tensor_copy(q_raw[:, pack, cols], psum_tile)

    # ---- Phase A: KV proj, V transpose, k rope, pack-0 q staging ----
    with (
        tc.tile_pool(name="psA", bufs=3, space="PSUM") as psA,
        tc.tile_pool(name="ptr", bufs=2, space="PSUM") as ptr,
    ):
        for ch in range(4):
            cols = slice(ch * 512, (ch + 1) * 512)
            pkv = psA.tile([128, 512], F32, tag="pj")
            for kb in range(NDB):
                nc.tensor.matmul(
                    pkv, wkv_sb[:, kb, :], xt_sb[:, kb, cols],
                    start=(kb == 0), stop=(kb == NDB - 1),
                )
            nc.vector.tensor_copy(k_raw[:, cols], pkv[0:64, :])
            nc.vector.tensor_copy(vt_sb[:, cols], pkv[64:128, :])
            for t in range(ch * 4, ch * 4 + 4):
                pt = ptr.tile([128, 64], BF16, tag="pjt")
                nc.tensor.transpose(
                    pt[:, 0:64], vt_sb[:, t * 128:(t + 1) * 128], id64
                )
                nc.vector.tensor_copy(vaug_sb[:, t, 0:64], pt[:, 0:64])
            if ch == 1 or ch == 3:
                half = ch // 2
                hcols = slice(half * 1024, (half + 1) * 1024)
                rope(kt_sb[0:64, hcols], k_raw[:, hcols], hcols, 0)
        for ch in range(4):
            pq = psA.tile([128, 512], F32, tag="pj")
            qproj(pq, 0, ch)

    # ---- Phase B: attention (+ pack-1 q proj injected into chunks 0-3) ----
    chunks = [(h, half) for h in range(HPC) for half in range(2)]

    def jit_rope(qc):
        h, half = chunks[qc]
        pack, row0 = h // 2, (h % 2) * 64
        cols = slice(half * 1024, (half + 1) * 1024)
        rope(qt_sb[0:64, h, cols], q_raw[row0:row0 + 64, pack, cols], cols, row0)

    def drain(qc, po):
        """Normalize chunk qc's PV psum -> ot_sb (and free po)."""
        h, half = chunks[qc]
        pack, row0 = h // 2, (h % 2) * 64
        cols = slice(half * 1024, (half + 1) * 1024)
        g0 = h * 16 + half * 8
        # denom row -> SBUF staging -> refold onto 128 partitions (natural
        # order; any bijection works since recip is elementwise and the DMA
        # back inverts it).
        row = dnrow[0:1, qc * 1024:(qc + 1) * 1024]
        nc.vector.tensor_copy(row, po[64:65, :, :].rearrange("p a b -> p (a b)"))
        nc.sync.dma_start(dn_sb[:, g0:g0 + 8], row)
        nc.vector.reciprocal(rc_sb[:, g0:g0 + 8], dn_sb[:, g0:g0 + 8])
        nc.sync.dma_start(
            rcrow[0:1, qc * 1024:(qc + 1) * 1024], rc_sb[:, g0:g0 + 8]
        )
        bc = bc_sb[:, qc % 2, :]
        nc.gpsimd.partition_broadcast(bc, rcrow[0:1, qc * 1024:(qc + 1) * 1024])
        nc.vector.tensor_mul(
            ot_sb[row0:row0 + 64, pack, cols],
            po[0:64, :, :].rearrange("p a b -> p (a b)"), bc
        )

    jit_rope(0)
    jit_rope(1)
    po_tiles = {}
    with (
        tc.tile_pool(name="psS", bufs=2, space="PSUM") as psS,
        tc.tile_pool(name="psPO", bufs=2, space="PSUM") as psPO,
    ):
        for qc, (h, half) in enumerate(chunks):
            po = psPO.tile([128, 8, 128], F32, tag="po")
            po_tiles[qc] = po
            first = True
            for kb in range(NKB):
                ps = psS.tile([128, 1024], F32, tag="s")
                for sh in range(2):
                    nc.tensor.matmul(
                        ps[:, sh * 512:(sh + 1) * 512],
                        kt_sb[:, kb * 128:(kb + 1) * 128],
                        qt_sb[:, h,
                              half * 1024 + sh * 512: half * 1024 + (sh + 1) * 512],
                        start=True, stop=True,
                    )
                p_t = ppool.tile([128, 1024], BF16, tag="p")
                nc.scalar.activation(p_t, ps, EXP, bias=0.0, scale=SCALE)
                for sh in range(2):
                    nc.tensor.matmul(
                        po[:, sh * 4:(sh + 1) * 4, :],
                        vaug_sb[:, kb, :],
                        p_t[:, sh * 512:(sh + 1) * 512],
                        start=(kb == 0), stop=(kb == NKB - 1),
                        skip_group_check=True,
                    )
                if first:
                    # previous chunk's drain ops go early on the DVE queue.
                    first = False
                    if qc >= 1:
                        drain(qc - 1, po_tiles.pop(qc - 1))
            if qc < 4:
                # pack-1 q projection for column chunk qc, using a psS slot
                # during the exp-bound steady state (shortens phase A).
                psq = psS.tile([128, 1024], F32, tag="s")
                qproj(psq[:, 0:512], 1, qc)
            if qc + 2 < NCH:
                # next-next chunk's rope, after the qproj that feeds it.
                jit_rope(qc + 2)
        drain(NCH - 1, po_tiles.pop(NCH - 1))

    # ---- Phase C: output projection out[q, :] = sum_pair O^T_pair.T @ Wo ----
    with tc.tile_pool(name="psOP", bufs=4, space="PSUM") as psOP:
        for qb in range(N // 128):
            for nchk in range(2):
                pt = psOP.tile([128, 512], F32, tag="po")
                for pair in range(2):
                    nc.tensor.matmul(
                        pt,
                        ot_sb[:, pair, qb * 128:(qb + 1) * 128],
                        wo_sb[:, pair, nchk * 512:(nchk + 1) * 512],
                        start=(pair == 0), stop=(pair == 1),
                    )
                st = stpool.tile([128, 512], BF16, tag="st")
                if (qb * 2 + nchk) % 2 == 0:
                    nc.vector.tensor_copy(st, pt)
                else:
                    nc.scalar.copy(st, pt)
                nc.sync.dma_start(
                    out[qb * 128:(qb + 1) * 128, nchk * 512:(nchk + 1) * 512], st
                )

    es.close()


def _rope_tables():
    inv_freq = 1.0 / (10000.0 ** (np.arange(0, D, 2, dtype=np.float64) / D))
    freqs = np.outer(np.arange(N, dtype=np.float64), inv_freq)  # [N, 32]
    cos_h = np.cos(freqs).T                                      # [32, N]
    sin_h = np.sin(freqs).T                                      # [32, N]
    # replicated over 128 partitions; sin blocks alternate [+sin; -sin] so
    # rope's swapped-row muls read matching base partitions (see rope()).
    cost = np.tile(cos_h, (4, 1)).astype(np.float32)
    sincat = np.concatenate(
        [sin_h, -sin_h, sin_h, -sin_h], 0
    ).astype(np.float32)
    return np.ascontiguousarray(cost), np.ascontiguousarray(sincat)


@functools.lru_cache(maxsize=1)
def _program():
    nc = bacc.Bacc(
        "TRN2", target_bir_lowering=False, debug=False, enable_asserts=False
    )
    io = {
        "xt": nc.dram_tensor("xt", [NDB, 128, N], BF16, kind="ExternalInput").ap(),
        "wq": nc.dram_tensor("wq", [NDB, 128, 256], BF16, kind="ExternalInput").ap(),
        "wkv": nc.dram_tensor("wkv", [NDB, 128, 128], BF16, kind="ExternalInput").ap(),
        "wo": nc.dram_tensor("wo", [2, 128, DIM], BF16, kind="ExternalInput").ap(),
        "cost": nc.dram_tensor("cost", [128, N], F32, kind="ExternalInput").ap(),
        "sincat": nc.dram_tensor("sincat", [128, N], F32, kind="ExternalInput").ap(),
        "out": nc.dram_tensor("out", [N, DIM], BF16, kind="ExternalOutput").ap(),
    }
    with tile.TileContext(nc) as tc:
        build_kernel(nc, tc, io)
    nc.compile()
    return nc


def make_in_maps(x, Wq, Wkv, Wo):
    import ml_dtypes

    bf16 = ml_dtypes.bfloat16
    cost, sincat = _rope_tables()
    in_maps = []
    for c in range(8):
        b, j = c // 4, c % 4
        xt = np.ascontiguousarray(x[b].T).reshape(NDB, 128, N)
        wq_c = np.ascontiguousarray(Wq[:, 256 * j:256 * (j + 1)]).reshape(
            NDB, 128, 256
        )
        wkv_c = np.ascontiguousarray(
            np.concatenate(
                [Wkv[:, 64 * j:64 * (j + 1)],
                 Wkv[:, 256 + 64 * j:256 + 64 * (j + 1)]],
                axis=1,
            )
        ).reshape(NDB, 128, 128)
        wo_c = np.ascontiguousarray(Wo[256 * j:256 * (j + 1), :]).reshape(
            2, 128, DIM
        )
        in_maps.append(
            {
                "xt": xt.astype(bf16),
                "wq": wq_c.astype(bf16),
                "wkv": wkv_c.astype(bf16),
                "wo": wo_c.astype(bf16),
                "cost": cost,
                "sincat": sincat,
            }
        )
    return in_maps


def _install_ntff_hook():
    """Register the axon NTFF profiling hook that this image's antenv lacks."""
    import types

    if "antenv.axon_hooks" in sys.modules:
        return
    try:
        sys.path.append("/root/.axon_site")
        from trn_agent_boot.trn_boot import _ntff_profile_via_ctypes

        hook = _ntff_profile_via_ctypes("/opt/axon/libaxon_pjrt.so")
    except Exception:
        hook = None
    finally:
        try:
            sys.path.remove("/root/.axon_site")
        except ValueError:
            pass
    mod = types.ModuleType("antenv.axon_hooks")
    mod.get_axon_ntff_profile_hook = lambda: hook
    mod.set_axon_ntff_profile_hook = lambda h: None
    sys.modules["antenv.axon_hooks"] = mod
    # artifact upload needs bucket credentials this container lacks
    import concourse.bass_utils as bu

    bu.upload_artifacts = lambda tmpdir: "local://" + str(tmpdir)


def kernel(x, Wq, Wkv, Wo, bo):
    from concourse.bass_utils import run_bass_kernel_spmd

    _install_ntff_hook()
    nc = _program()
    in_maps = make_in_maps(x, Wq, Wkv, Wo)
    trace = bool(os.environ.get("KERNEL_TRACE"))
    res = run_bass_kernel_spmd(
        nc, in_maps, list(range(8)), trace=trace
    )
    LAST_RESULTS["res"] = res
    full = np.zeros((B, N, DIM), np.float32)
    for c in range(8):
        full[c // 4] += res.results[c]["out"].astype(np.float32)
    full += bo.astype(np.float32)
    return full


# revision 23
# speedup vs baseline: 1.6071x; 1.0083x over previous
"""GQA attention (16 Q heads / 4 KV heads, RoPE, n=2048, d=64) on 8 trn2 cores.

Sharding: core c = (batch b=c//4, kv-group j=c%4). Each core owns 4 query
heads sharing one KV head, computes its partial output projection
(O_heads @ Wo_rows), and the host sums the 4 partials per batch.

v3 design (exp-bound pipeline, bf16):
  - bf16 matmul operands everywhere (fp16 measured ~2.7x slower on the real
    PE despite the cost model); S/PV matmuls zero-padded to 128-partition /
    128-col shapes (odd shapes also measured slow).
  - Phase A: KV + pack-0 Q projections -> stage RAW q/k in SBUF; RoPE
    applied just-in-time per attention chunk from SBUF on the DVE.
    Pack-1 Q projections are injected into attention chunks 0-3 (psS slot
    steal) to shorten the startup ramp.
  - Phase B: 8 chunks (head, 1024-query half). Per key block: S matmul
    (2x 512-col) -> exp on ACT engine [128,1024] -> PV accumulate.
    ACT exp is the roofline (~131k cols * 0.83ns = 109us + op overheads).
    PSUM: S pool 2x2 banks + PO pool 2x2 banks = 8.
  - Softmax denominators via ones-column of V_aug land on PSUM row 64;
    reciprocal'd in a [128, 8] transposed layout (DMA round trip) instead
    of single-partition reciprocals, broadcast multiply on drain.
  - Phase C: output projection, bf16 output DMA streamed per tile; host
    sums the per-core partials in fp32.
"""

import os
import sys
import functools

import numpy as np

sys.path.insert(0, "/opt/trn_rl_repo")

import concourse.bass as bass  # noqa: E402
import concourse.bacc as bacc  # noqa: E402
import concourse.tile as tile  # noqa: E402
import concourse.mybir as mybir  # noqa: E402
from concourse.masks import make_identity  # noqa: E402

F32 = mybir.dt.float32
BF16 = mybir.dt.bfloat16
EXP = mybir.ActivationFunctionType.Exp

B, N, DIM = 2, 2048, 1024
HEADS, KVH, D = 16, 4, 64
HPC = HEADS // KVH          # q heads per core = 4
SCALE = D ** -0.5           # 1/8
NKB = N // 128              # 16 key blocks
NDB = DIM // 128            # 8 contraction blocks for projections
NCH = 8                     # attention chunks: (head, half) -> 1024 queries

LAST_RESULTS = {}           # test.py introspection


def build_kernel(nc, tc, io):
    from contextlib import ExitStack

    xt, wq, wkv, wo = io["xt"], io["wq"], io["wkv"], io["wo"]
    cost, sincat, out = io["cost"], io["sincat"], io["out"]

    es = ExitStack()
    consts = es.enter_context(tc.tile_pool(name="consts", bufs=1))
    act = es.enter_context(tc.tile_pool(name="act", bufs=1))
    ropetmp = es.enter_context(tc.tile_pool(name="ropetmp", bufs=2))
    ppool = es.enter_context(tc.tile_pool(name="ppool", bufs=3))
    stpool = es.enter_context(tc.tile_pool(name="stpool", bufs=8))

    # --- constants / weights in SBUF ---
    wq_sb = consts.tile([128, NDB, 256], BF16, tag="wq")
    wkv_sb = consts.tile([128, NDB, 128], BF16, tag="wkv")
    wo_sb = consts.tile([128, 2, DIM], BF16, tag="wo")
    # cos/sin replicated over all 128 partitions; sin table stores
    # [+sin; -sin] per 64-row block so every rope mul reads its two SBUF
    # inputs at matching base partitions (BIR constraint).
    cos_sb = consts.tile([128, N], F32, tag="cos")
    sin_sb = consts.tile([128, N], F32, tag="sin")
    id64 = consts.tile([64, 64], BF16, tag="id")
    make_identity(nc, id64)

    # --- persistent activations ---
    xt_sb = act.tile([128, NDB, N], BF16, tag="xt")
    k_raw = act.tile([64, N], BF16, tag="kraw")
    vt_sb = act.tile([64, N], BF16, tag="vt")
    q_raw = act.tile([128, 2, N], BF16, tag="qraw")       # [2 heads x 64, pack, n]
    kt_sb = act.tile([128, N], BF16, tag="kt")            # rows 64:128 zero
    qt_sb = act.tile([128, HPC, N], BF16, tag="qt")       # rows 64:128 zero
    vaug_sb = act.tile([128, NKB, 128], BF16, tag="vaug")  # v | ones | zeros
    ot_sb = act.tile([128, 2, N], BF16, tag="ot")         # normalized O^T, 2 packs
    dnrow = act.tile([1, 2, 1024], F32, tag="dnrow")      # denom rows staging
    dn_sb = act.tile([128, 64], F32, tag="dn")            # denoms, refolded
    rc_sb = act.tile([128, 64], F32, tag="rc")            # 1/denoms
    rcrow = act.tile([1, 2, 1024], F32, tag="rcrow")      # 1/denom row form
    bc_sb = act.tile([64, 2, 1024], F32, tag="bc")        # broadcast 1/denom

    nc.gpsimd.memset(kt_sb[64:128, :], 0.0)
    nc.gpsimd.memset(qt_sb[64:128, :, :], 0.0)
    nc.gpsimd.memset(vaug_sb[:, :, 64:65], 1.0)
    nc.gpsimd.memset(vaug_sb[:, :, 65:128], 0.0)

    # DMA issue order tracks first use: wkv + xt ch0/ch1 feed the first
    # projections; wo (phase C only) goes last.
    def xt_dma(ch):
        for kb in range(NDB):
            nc.sync.dma_start(
                xt_sb[:, kb, ch * 512:(ch + 1) * 512],
                xt[kb, :, ch * 512:(ch + 1) * 512],
            )

    nc.sync.dma_start(wkv_sb, wkv.transpose([1, 0, 2]))
    xt_dma(0)
    xt_dma(1)
    nc.sync.dma_start(cos_sb[0:64, :], cost)
    nc.sync.dma_start(cos_sb[64:128, :], cost)
    nc.sync.dma_start(sin_sb[0:64, :], sincat)
    nc.sync.dma_start(sin_sb[64:128, :], sincat)
    nc.sync.dma_start(wq_sb, wq.transpose([1, 0, 2]))
    xt_dma(2)
    xt_dma(3)
    nc.sync.dma_start(wo_sb, wo.transpose([1, 0, 2]))

    def rope(dst, src, cols, row0):
        """dst[64, w] <- RoPE(src bf16 SBUF at base partition row0)."""
        w = cols.stop - cols.start
        t1f = ropetmp.tile([64, 1024], F32, tag="t1", name="t1f")
        t2f = ropetmp.tile([64, 1024], F32, tag="t2", name="t2f")
        t1, t2 = t1f[:, 0:w], t2f[:, 0:w]
        nc.vector.tensor_mul(t1, src, cos_sb[row0:row0 + 64, cols])
        nc.vector.tensor_mul(
            t2[0:32, :], src[32:64, :], sin_sb[row0 + 32:row0 + 64, cols]
        )
        nc.vector.tensor_mul(
            t2[32:64, :], src[0:32, :], sin_sb[row0:row0 + 32, cols]
        )
        nc.vector.tensor_add(dst, t1, t2)

    def qproj(psum_tile, pack, ch):
        cols = slice(ch * 512, (ch + 1) * 512)
        for kb in range(NDB):
            nc.tensor.matmul(
                psum_tile, wq_sb[:, kb, pack * 128:(pack + 1) * 128],
                xt_sb[:, kb, cols],
                start=(kb == 0), stop=(kb == NDB - 1),
            )
        nc.vector.tensor_copy(q_raw[:, pack, cols], psum_tile)

    # ---- Phase A: KV proj, V transpose, k rope, pack-0 q staging ----
    with (
        tc.tile_pool(name="psA", bufs=3, space="PSUM") as psA,
        tc.tile_pool(name="ptr", bufs=2, space="PSUM") as ptr,
    ):
        for ch in range(4):
            cols = slice(ch * 512, (ch + 1) * 512)
            pkv = psA.tile([128, 512], F32, tag="pj")
            for kb in range(NDB):
                nc.tensor.matmul(
                    pkv, wkv_sb[:, kb, :], xt_sb[:, kb, cols],
                    start=(kb == 0), stop=(kb == NDB - 1),
                )
            nc.vector.tensor_copy(k_raw[:, cols], pkv[0:64, :])
            nc.vector.tensor_copy(vt_sb[:, cols], pkv[64:128, :])
            for t in range(ch * 4, ch * 4 + 4):
                pt = ptr.tile([128, 64], BF16, tag="pjt")
                nc.tensor.transpose(
                    pt[:, 0:64], vt_sb[:, t * 128:(t + 1) * 128], id64
                )
                nc.vector.tensor_copy(vaug_sb[:, t, 0:64], pt[:, 0:64])
            if ch == 1 or ch == 3:
                half = ch // 2
                hcols = slice(half * 1024, (half + 1) * 1024)
                rope(kt_sb[0:64, hcols], k_raw[:, hcols], hcols, 0)
        for ch in range(4):
            pq = psA.tile([128, 512], F32, tag="pj")
            qproj(pq, 0, ch)

    # ---- Phase B: attention (+ pack-1 q proj injected into chunks 0-3) ----
    chunks = [(h, half) for h in range(HPC) for half in range(2)]

    def jit_rope(qc):
        h, half = chunks[qc]
        pack, row0 = h // 2, (h % 2) * 64
        cols = slice(half * 1024, (half + 1) * 1024)
        rope(qt_sb[0:64, h, cols], q_raw[row0:row0 + 64, pack, cols], cols, row0)

    def drain(qc, po):
        """Normalize chunk qc's PV psum -> ot_sb (and free po)."""
        h, half = chunks[qc]
        pack, row0 = h // 2, (h % 2) * 64
        cols = slice(half * 1024, (half + 1) * 1024)
        g0 = h * 16 + half * 8
        # denom row -> SBUF staging -> refold onto 128 partitions (natural
        # order; any bijection works since recip is elementwise and the DMA
        # back inverts it).
        row = dnrow[0:1, qc % 2, :]
        nc.vector.tensor_copy(row, po[64:65, :, :].rearrange("p a b -> p (a b)"))
        nc.sync.dma_start(dn_sb[:, g0:g0 + 8], row)
        nc.vector.reciprocal(rc_sb[:, g0:g0 + 8], dn_sb[:, g0:g0 + 8])
        rrow = rcrow[0:1, qc % 2, :]
        nc.sync.dma_start(rrow, rc_sb[:, g0:g0 + 8])
        bc = bc_sb[:, qc % 2, :]
        nc.gpsimd.partition_broadcast(bc, rrow)
        nc.vector.tensor_mul(
            ot_sb[row0:row0 + 64, pack, cols],
            po[0:64, :, :].rearrange("p a b -> p (a b)"), bc
        )

    jit_rope(0)
    jit_rope(1)
    with (
        tc.tile_pool(name="psS", bufs=2, space="PSUM") as psS,
        tc.tile_pool(name="psPO", bufs=2, space="PSUM") as psPO,
    ):
        # po tiles allocated one chunk early so the next chunk's (currently
        # idle) buffer can host the injected pack-1 q-projection accumulation.
        po_tiles = {0: psPO.tile([128, 8, 128], F32, tag="po", name="po_t0")}
        for qc, (h, half) in enumerate(chunks):
            if qc + 1 < NCH:
                po_tiles[qc + 1] = psPO.tile(
                    [128, 8, 128], F32, tag="po", name=f"po_t{qc + 1}"
                )
            po = po_tiles[qc]
            psq = po_tiles[qc + 1][:, 0:4, :] if qc < 4 else None
            first = True
            for kb in range(NKB):
                ps = psS.tile([128, 1024], F32, tag="s")
                for sh in range(2):
                    nc.tensor.matmul(
                        ps[:, sh * 512:(sh + 1) * 512],
                        kt_sb[:, kb * 128:(kb + 1) * 128],
                        qt_sb[:, h,
                              half * 1024 + sh * 512: half * 1024 + (sh + 1) * 512],
                        start=True, stop=True,
                    )
                p_t = ppool.tile([128, 1024], BF16, tag="p")
                nc.scalar.activation(p_t, ps, EXP, bias=0.0, scale=SCALE)
                for sh in range(2):
                    nc.tensor.matmul(
                        po[:, sh * 4:(sh + 1) * 4, :],
                        vaug_sb[:, kb, :],
                        p_t[:, sh * 512:(sh + 1) * 512],
                        start=(kb == 0), stop=(kb == NKB - 1),
                        skip_group_check=True,
                    )
                if qc < 4 and kb >= 12:
                    # pack-1 q projection, two contraction steps per kb,
                    # accumulating in the idle po buffer (fully drained of
                    # the previous chunk by now). Keeps the exp pipeline fed.
                    for dkb in (2 * (kb - 12), 2 * (kb - 12) + 1):
                        nc.tensor.matmul(
                            psq, wq_sb[:, dkb, 128:256],
                            xt_sb[:, dkb, qc * 512:(qc + 1) * 512],
                            start=(dkb == 0), stop=(dkb == NDB - 1),
                            skip_group_check=True,
                        )
                if first:
                    # previous chunk's drain ops go early on the DVE queue.
                    first = False
                    if qc >= 1:
                        drain(qc - 1, po_tiles.pop(qc - 1))
            if qc < 4:
                nc.vector.tensor_copy(
                    q_raw[:, 1, qc * 512:(qc + 1) * 512], psq
                )
            if qc + 2 < NCH:
                # next-next chunk's rope, after the qproj that feeds it.
                jit_rope(qc + 2)
        drain(NCH - 1, po_tiles.pop(NCH - 1))

    # ---- Phase C: output projection out[q, :] = sum_pair O^T_pair.T @ Wo ----
    with tc.tile_pool(name="psOP", bufs=8, space="PSUM") as psOP:
        for qb in range(N // 128):
            for nchk in range(2):
                pt = psOP.tile([128, 512], F32, tag="po")
                for pair in range(2):
                    nc.tensor.matmul(
                        pt,
                        ot_sb[:, pair, qb * 128:(qb + 1) * 128],
                        wo_sb[:, pair, nchk * 512:(nchk + 1) * 512],
                        start=(pair == 0), stop=(pair == 1),
                    )
                st = stpool.tile([128, 512], BF16, tag="st")
                if (qb * 2 + nchk) % 2 == 0:
                    nc.vector.tensor_copy(st, pt)
                else:
                    nc.scalar.copy(st, pt)
                nc.sync.dma_start(
                    out[qb * 128:(qb + 1) * 128, nchk * 512:(nchk + 1) * 512], st
                )

    es.close()


def _rope_tables():
    inv_freq = 1.0 / (10000.0 ** (np.arange(0, D, 2, dtype=np.float64) / D))
    freqs = np.outer(np.arange(N, dtype=np.float64), inv_freq)  # [N, 32]
    cos_h = np.cos(freqs).T                                      # [32, N]
    sin_h = np.sin(freqs).T                                      # [32, N]
    # replicated over 128 partitions; sin blocks alternate [+sin; -sin] so
    # rope's swapped-row muls read matching base partitions (see rope()).
    cost = np.tile(cos_h, (2, 1)).astype(np.float32)
    sincat = np.concatenate([sin_h, -sin_h], 0).astype(np.float32)
    return np.ascontiguousarray(cost), np.ascontiguousarray(sincat)


@functools.lru_cache(maxsize=1)
def _program():
    nc = bacc.Bacc(
        "TRN2", target_bir_lowering=False, debug=False, enable_asserts=False
    )
    io = {
        "xt": nc.dram_tensor("xt", [NDB, 128, N], BF16, kind="ExternalInput").ap(),
        "wq": nc.dram_tensor("wq", [NDB, 128, 256], BF16, kind="ExternalInput").ap(),
        "wkv": nc.dram_tensor("wkv", [NDB, 128, 128], BF16, kind="ExternalInput").ap(),
        "wo": nc.dram_tensor("wo", [2, 128, DIM], BF16, kind="ExternalInput").ap(),
        "cost": nc.dram_tensor("cost", [64, N], F32, kind="ExternalInput").ap(),
        "sincat": nc.dram_tensor("sincat", [64, N], F32, kind="ExternalInput").ap(),
        "out": nc.dram_tensor("out", [N, DIM], BF16, kind="ExternalOutput").ap(),
    }
    with tile.TileContext(nc) as tc:
        build_kernel(nc, tc, io)
    nc.compile()
    return nc


def make_in_maps(x, Wq, Wkv, Wo):
    import ml_dtypes

    bf16 = ml_dtypes.bfloat16
    cost, sincat = _rope_tables()
    in_maps = []
    for c in range(8):
        b, j = c // 4, c % 4
        xt = np.ascontiguousarray(x[b].T).reshape(NDB, 128, N)
        wq_c = np.ascontiguousarray(Wq[:, 256 * j:256 * (j + 1)]).reshape(
            NDB, 128, 256
        )
        wkv_c = np.ascontiguousarray(
            np.concatenate(
                [Wkv[:, 64 * j:64 * (j + 1)],
                 Wkv[:, 256 + 64 * j:256 + 64 * (j + 1)]],
                axis=1,
            )
        ).reshape(NDB, 128, 128)
        wo_c = np.ascontiguousarray(Wo[256 * j:256 * (j + 1), :]).reshape(
            2, 128, DIM
        )
        in_maps.append(
            {
                "xt": xt.astype(bf16),
                "wq": wq_c.astype(bf16),
                "wkv": wkv_c.astype(bf16),
                "wo": wo_c.astype(bf16),
                "cost": cost,
                "sincat": sincat,
            }
        )
    return in_maps


def _install_ntff_hook():
    """Register the axon NTFF profiling hook that this image's antenv lacks."""
    import types

    if "antenv.axon_hooks" in sys.modules:
        return
    try:
        sys.path.append("/root/.axon_site")
        from trn_agent_boot.trn_boot import _ntff_profile_via_ctypes

        hook = _ntff_profile_via_ctypes("/opt/axon/libaxon_pjrt.so")
    except Exception:
        hook = None
    finally:
        try:
            sys.path.remove("/root/.axon_site")
        except ValueError:
            pass
    mod = types.ModuleType("antenv.axon_hooks")
    mod.get_axon_ntff_profile_hook = lambda: hook
    mod.set_axon_ntff_profile_hook = lambda h: None
    sys.modules["antenv.axon_hooks"] = mod
    # artifact upload needs bucket credentials this container lacks
    import concourse.bass_utils as bu

    bu.upload_artifacts = lambda tmpdir: "local://" + str(tmpdir)


def kernel(x, Wq, Wkv, Wo, bo):
    from concourse.bass_utils import run_bass_kernel_spmd

    _install_ntff_hook()
    nc = _program()
    in_maps = make_in_maps(x, Wq, Wkv, Wo)
    trace = bool(os.environ.get("KERNEL_TRACE"))
    res = run_bass_kernel_spmd(
        nc, in_maps, list(range(8)), trace=trace
    )
    LAST_RESULTS["res"] = res
    full = np.zeros((B, N, DIM), np.float32)
    for c in range(8):
        full[c // 4] += res.results[c]["out"].astype(np.float32)
    full += bo.astype(np.float32)
    return full
